# revision 5
# baseline (speedup 1.0000x reference)
"""Trainium2 Bass kernel for nn_NaiveBayes (Gaussian naive-Bayes relation scorer).

Reference computes, for x = concat(sbjs, objs) [B, 2D]:
    out[b, r] = sum_d[ -0.5*((x_bd - mu_rd)/sig_rd)^2 - log(sig_rd) - LOG_SQRT_2PI ]
                + prior_r * 2D

Expanded into a matmul (per relation r, feature d):
    out[b, r] = sum_d x_bd * Wx[d, r] + sum_d (x_bd^2) * Wsq[d, r] + c_r
      Wx[d, r]  = mu_rd / sig_rd^2
      Wsq[d, r] = -0.5 / sig_rd^2
      c_r       = sum_d(-0.5*mu^2/sig^2 - log sig - LOG_SQRT_2PI) + prior_r * 2D

Sharding: data-parallel over batch: 4096 rows -> 8 cores x 512 rows.
mus/sigmas/priors are folded host-side into W [1024, 128] and c [128] and
replicated to all cores.  X is pre-transposed host-side to [d, b] layout
(f32 DMA-transpose is unsupported on TRN2).  Each core computes
out^T [128 r, 512 b] with 8 accumulating PE matmuls (K = 8 x 128 chunks:
x-stream then x^2-stream), squares computed on DVE, c added during PSUM
eviction.  Host transposes + concatenates the 8 blocks.
"""

import numpy as np

import concourse.bacc as bacc
import concourse.tile as tile
from concourse import mybir
from concourse.bass_utils import run_bass_kernel_spmd

NCORES = 8
B = 4096
D = 256
TWO_D = 2 * D  # 512 features
R = 128  # relations
BPC = B // NCORES  # 512 batch rows per core
KCH = TWO_D // 128  # 4 feature chunks of 128
LOG_SQRT_2PI = 0.9189385332046727

F32 = mybir.dt.float32

_NC_CACHE = {}


def _build_nc(mm_dt):
    nc = bacc.Bacc("TRN2", target_bir_lowering=False, debug=False)

    xt = nc.dram_tensor("xt", [TWO_D, BPC], mm_dt, kind="ExternalInput")
    w = nc.dram_tensor("w", [2 * TWO_D, R], mm_dt, kind="ExternalInput")
    cvec = nc.dram_tensor("cvec", [R, 1], F32, kind="ExternalInput")
    out = nc.dram_tensor("out", [R, BPC], F32, kind="ExternalOutput")

    w_r = w.ap().rearrange("(k p) r -> p k r", p=128)  # [128, 8, 128]
    xt_r = xt.ap().rearrange("(k p) b -> p k b", p=128)  # [128, 4, 512]

    with tile.TileContext(nc) as tc:
        with (
            tc.tile_pool(name="const", bufs=1) as const,
            tc.tile_pool(name="data", bufs=1) as data,
            tc.tile_pool(name="psum", bufs=1, space="PSUM") as psum,
        ):
            c_sb = const.tile([R, 1], F32)
            nc.sync.dma_start(c_sb[:], cvec.ap())

            w_sb = const.tile([128, 2 * KCH, R], mm_dt)
            # x-stream coefficients (chunks 0..3) first: first matmuls need them
            nc.sync.dma_start(w_sb[:, 0:KCH, :], w_r[:, 0:KCH, :])

            xt_sb = data.tile([128, KCH, BPC], mm_dt)
            sq_sb = data.tile([128, KCH, BPC], mm_dt)
            for k in range(KCH):
                nc.sync.dma_start(xt_sb[:, k, :], xt_r[:, k, :])
            # x^2-stream coefficients (chunks 4..7)
            nc.sync.dma_start(w_sb[:, KCH : 2 * KCH, :], w_r[:, KCH : 2 * KCH, :])

            for k in range(KCH):
                nc.vector.tensor_mul(sq_sb[:, k, :], xt_sb[:, k, :], xt_sb[:, k, :])

            ps = psum.tile([R, BPC], F32)
            for k in range(KCH):
                nc.tensor.matmul(
                    ps[:],
                    w_sb[:, k, :],
                    xt_sb[:, k, :],
                    start=(k == 0),
                    stop=False,
                )
            for k in range(KCH):
                nc.tensor.matmul(
                    ps[:],
                    w_sb[:, KCH + k, :],
                    sq_sb[:, k, :],
                    start=False,
                    stop=(k == KCH - 1),
                )

            out_sb = data.tile([R, BPC], F32)
            nc.vector.tensor_scalar_add(out_sb[:], ps[:], c_sb[:])
            nc.sync.dma_start(out.ap(), out_sb[:])

    nc.compile()
    return nc


def _prepare(sbjs, objs, mus, sigmas, relation_priors):
    """Host-side parameter folding + batch sharding. Returns per-core in_maps."""
    mus64 = mus.astype(np.float64)
    sig64 = sigmas.astype(np.float64)
    sig2 = sig64 * sig64
    wx = mus64 / sig2  # [R, 2D]
    wsq = -0.5 / sig2  # [R, 2D]
    c = (
        (-0.5 * mus64 * mus64 / sig2 - np.log(sig64) - LOG_SQRT_2PI).sum(axis=1)
        + relation_priors.astype(np.float64) * TWO_D
    )

    w_full = np.concatenate([wx.T, wsq.T], axis=0).astype(np.float32)  # [1024, R]
    w_full = np.ascontiguousarray(w_full)
    c32 = np.ascontiguousarray(c.astype(np.float32).reshape(R, 1))

    x = np.concatenate([sbjs, objs], axis=1)  # [B, 2D] f32
    xt_full = np.ascontiguousarray(x.T.astype(np.float32))  # [2D, B]

    in_maps = []
    for i in range(NCORES):
        xt_i = np.ascontiguousarray(xt_full[:, i * BPC : (i + 1) * BPC])
        in_maps.append({"xt": xt_i, "w": w_full, "cvec": c32})
    return in_maps


def run(sbjs, objs, mus, sigmas, relation_priors, mm_dt=F32, **run_kwargs):
    """Build (cached), run on 8 cores, gather. Returns (out [B, R] f32, results)."""
    key = str(mm_dt)
    if key not in _NC_CACHE:
        _NC_CACHE[key] = _build_nc(mm_dt)
    nc = _NC_CACHE[key]

    in_maps = _prepare(sbjs, objs, mus, sigmas, relation_priors)
    res = run_bass_kernel_spmd(nc, in_maps, core_ids=list(range(NCORES)), **run_kwargs)

    out = np.empty((B, R), dtype=np.float32)
    for i in range(NCORES):
        out[i * BPC : (i + 1) * BPC, :] = res.results[i]["out"].T
    return out, res


def kernel(sbjs, objs, mus, sigmas, relation_priors):
    out, _ = run(sbjs, objs, mus, sigmas, relation_priors)
    return out


if __name__ == "__main__":
    rng = np.random.default_rng(0)
    ins = {
        "sbjs": rng.standard_normal((B, D), dtype=np.float32),
        "objs": rng.standard_normal((B, D), dtype=np.float32),
        "mus": rng.standard_normal((R, TWO_D), dtype=np.float32),
        "sigmas": np.abs(rng.standard_normal((R, TWO_D), dtype=np.float32)) + 1.0,
        "relation_priors": rng.standard_normal((R,), dtype=np.float32),
    }
    out = kernel(**ins)
    print("out", out.shape, out.dtype, float(np.abs(out).max()))


# revision 9
# speedup vs baseline: 1.0342x; 1.0342x over previous
"""Trainium2 Bass kernel for nn_NaiveBayes (Gaussian naive-Bayes relation scorer).

Reference computes, for x = concat(sbjs, objs) [B, 2D]:
    out[b, r] = sum_d[ -0.5*((x_bd - mu_rd)/sig_rd)^2 - log(sig_rd) - LOG_SQRT_2PI ]
                + prior_r * 2D

Expanded into a matmul (per relation r, feature d):
    out[b, r] = sum_d x_bd * Wx[d, r] + sum_d (x_bd^2) * Wsq[d, r] + c_r
      Wx[d, r]  = mu_rd / sig_rd^2
      Wsq[d, r] = -0.5 / sig_rd^2
      c_r       = sum_d(-0.5*mu^2/sig^2 - log sig - LOG_SQRT_2PI) + prior_r * 2D

Sharding: data-parallel over batch: 4096 rows -> 8 cores x 512 rows.
mus/sigmas/priors are folded host-side into W [1024, 128] and c [128] and
replicated to all cores.  X is pre-transposed host-side to [d, b] layout
(f32 DMA-transpose is unsupported on TRN2).  Each core computes
out^T [128 r, 512 b] with 8 accumulating PE matmuls (K = 8 x 128 chunks:
x-stream then x^2-stream), squares computed on DVE, c added during PSUM
eviction.  Host transposes + concatenates the 8 blocks.
"""

import numpy as np

import concourse.bacc as bacc
import concourse.tile as tile
from concourse import mybir
from concourse.bass_utils import run_bass_kernel_spmd

NCORES = 8
B = 4096
D = 256
TWO_D = 2 * D  # 512 features
R = 128  # relations
BPC = B // NCORES  # 512 batch rows per core
KCH = TWO_D // 128  # 4 feature chunks of 128
LOG_SQRT_2PI = 0.9189385332046727

F32 = mybir.dt.float32

N_WARMUP = 10

_NC_CACHE = {}


def _build_nc(mm_dt):
    nc = bacc.Bacc("TRN2", target_bir_lowering=False, debug=False)

    xt = nc.dram_tensor("xt", [TWO_D, BPC], mm_dt, kind="ExternalInput")
    w = nc.dram_tensor("w", [2 * TWO_D, R], mm_dt, kind="ExternalInput")
    cvec = nc.dram_tensor("cvec", [R, 1], F32, kind="ExternalInput")
    out = nc.dram_tensor("out", [R, BPC], F32, kind="ExternalOutput")

    w_r = w.ap().rearrange("(k p) r -> p k r", p=128)  # [128, 8, 128]
    xt_r = xt.ap().rearrange("(k p) b -> p k b", p=128)  # [128, 4, 512]

    with tile.TileContext(nc) as tc:
        with (
            tc.tile_pool(name="const", bufs=1) as const,
            tc.tile_pool(name="data", bufs=1) as data,
            tc.tile_pool(name="psum", bufs=1, space="PSUM") as psum,
            tc.tile_pool(name="wpsum", bufs=1, space="PSUM") as wpsum_pool,
        ):
            # Input DMAs spread over both HWDGE queues (SP=sync, ACT=scalar)
            # so transfers overlap instead of serializing on one ring.
            xt_sb = data.tile([128, KCH, BPC], mm_dt)
            sq_sb = data.tile([128, KCH, BPC], mm_dt)
            w_sb = const.tile([128, 2 * KCH, R], mm_dt)
            c_sb = const.tile([R, 1], F32)

            for k in range(KCH):
                nc.sync.dma_start(xt_sb[:, k, :], xt_r[:, k, :])
            # x-stream coefficients (chunks 0..3) first: first matmuls need them
            nc.scalar.dma_start(w_sb[:, 0:KCH, :], w_r[:, 0:KCH, :])
            # x^2-stream coefficients (chunks 4..7)
            nc.scalar.dma_start(w_sb[:, KCH : 2 * KCH, :], w_r[:, KCH : 2 * KCH, :])
            nc.scalar.dma_start(c_sb[:], cvec.ap())

            # PE warmup: the HAM clock gate holds the PE at 1.2 GHz until it
            # has been busy ~3.4us. Dummy matmuls on a memset tile during the
            # DMA wait bring it to 2.4 GHz before the real matmuls issue.
            warm = const.tile([128, 128], F32)
            nc.gpsimd.memset(warm[:], 0.0)
            wps = wpsum_pool.tile([1, 128], F32)
            for _ in range(N_WARMUP):
                nc.tensor.matmul(wps[:], warm[:, 0:1], warm[:], start=True, stop=True)

            for k in range(KCH):
                nc.vector.tensor_mul(sq_sb[:, k, :], xt_sb[:, k, :], xt_sb[:, k, :])

            ps = psum.tile([R, BPC], F32)
            for k in range(KCH):
                nc.tensor.matmul(
                    ps[:],
                    w_sb[:, k, :],
                    xt_sb[:, k, :],
                    start=(k == 0),
                    stop=False,
                )
            for k in range(KCH):
                nc.tensor.matmul(
                    ps[:],
                    w_sb[:, KCH + k, :],
                    sq_sb[:, k, :],
                    start=False,
                    stop=(k == KCH - 1),
                )

            out_sb = data.tile([R, BPC], F32)
            nc.vector.tensor_scalar_add(out_sb[:], ps[:], c_sb[:])
            nc.sync.dma_start(out.ap(), out_sb[:])

    nc.compile()
    return nc


def _prepare(sbjs, objs, mus, sigmas, relation_priors):
    """Host-side parameter folding + batch sharding. Returns per-core in_maps."""
    mus64 = mus.astype(np.float64)
    sig64 = sigmas.astype(np.float64)
    sig2 = sig64 * sig64
    wx = mus64 / sig2  # [R, 2D]
    wsq = -0.5 / sig2  # [R, 2D]
    c = (
        (-0.5 * mus64 * mus64 / sig2 - np.log(sig64) - LOG_SQRT_2PI).sum(axis=1)
        + relation_priors.astype(np.float64) * TWO_D
    )

    w_full = np.concatenate([wx.T, wsq.T], axis=0).astype(np.float32)  # [1024, R]
    w_full = np.ascontiguousarray(w_full)
    c32 = np.ascontiguousarray(c.astype(np.float32).reshape(R, 1))

    x = np.concatenate([sbjs, objs], axis=1)  # [B, 2D] f32
    xt_full = np.ascontiguousarray(x.T.astype(np.float32))  # [2D, B]

    in_maps = []
    for i in range(NCORES):
        xt_i = np.ascontiguousarray(xt_full[:, i * BPC : (i + 1) * BPC])
        in_maps.append({"xt": xt_i, "w": w_full, "cvec": c32})
    return in_maps


def run(sbjs, objs, mus, sigmas, relation_priors, mm_dt=F32, **run_kwargs):
    """Build (cached), run on 8 cores, gather. Returns (out [B, R] f32, results)."""
    key = str(mm_dt)
    if key not in _NC_CACHE:
        _NC_CACHE[key] = _build_nc(mm_dt)
    nc = _NC_CACHE[key]

    in_maps = _prepare(sbjs, objs, mus, sigmas, relation_priors)
    res = run_bass_kernel_spmd(nc, in_maps, core_ids=list(range(NCORES)), **run_kwargs)

    out = np.empty((B, R), dtype=np.float32)
    for i in range(NCORES):
        out[i * BPC : (i + 1) * BPC, :] = res.results[i]["out"].T
    return out, res


def kernel(sbjs, objs, mus, sigmas, relation_priors):
    out, _ = run(sbjs, objs, mus, sigmas, relation_priors)
    return out


if __name__ == "__main__":
    rng = np.random.default_rng(0)
    ins = {
        "sbjs": rng.standard_normal((B, D), dtype=np.float32),
        "objs": rng.standard_normal((B, D), dtype=np.float32),
        "mus": rng.standard_normal((R, TWO_D), dtype=np.float32),
        "sigmas": np.abs(rng.standard_normal((R, TWO_D), dtype=np.float32)) + 1.0,
        "relation_priors": rng.standard_normal((R,), dtype=np.float32),
    }
    out = kernel(**ins)
    print("out", out.shape, out.dtype, float(np.abs(out).max()))


# revision 10
# speedup vs baseline: 1.1333x; 1.0958x over previous
"""Trainium2 Bass kernel for nn_NaiveBayes (Gaussian naive-Bayes relation scorer).

Reference computes, for x = concat(sbjs, objs) [B, 2D]:
    out[b, r] = sum_d[ -0.5*((x_bd - mu_rd)/sig_rd)^2 - log(sig_rd) - LOG_SQRT_2PI ]
                + prior_r * 2D

Expanded into a matmul (per relation r, feature d):
    out[b, r] = sum_d x_bd * Wx[d, r] + sum_d (x_bd^2) * Wsq[d, r] + c_r
      Wx[d, r]  = mu_rd / sig_rd^2
      Wsq[d, r] = -0.5 / sig_rd^2
      c_r       = sum_d(-0.5*mu^2/sig^2 - log sig - LOG_SQRT_2PI) + prior_r * 2D

Sharding: data-parallel over batch: 4096 rows -> 8 cores x 512 rows.
mus/sigmas/priors fold host-side into W and c, replicated to all cores.

The x / W streams ship as fp16: fp16's 10 mantissa bits match what the PE's
fp32r (TF32) mode keeps anyway (measured 1.40e-5 vs 1.44e-5 scale-relative
absmax), at half the HBM bytes and full PE rate. Accumulation is fp32 PSUM;
c is added in fp32.

Host pre-swizzles both streams into the exact SBUF layout (partition-major,
[128, chunk*free]) so every DMA is a contiguous line-rate copy; X is also
pre-transposed to [d, b] (f32/fp16 DMA-transpose is unsupported / 2-byte-only
and this is free on the host during sharding). Each core computes
out^T [128 r, 512 b]: 8 accumulating PE matmuls (K = 8 x 128 chunks:
x-stream then x^2-stream), squares on DVE, c added during PSUM eviction in
two halves overlapped with the two output DMAs on separate HWDGE queues.
Host transposes + concatenates the 8 blocks.
"""

import numpy as np

import concourse.bacc as bacc
import concourse.tile as tile
from concourse import mybir
from concourse.bass_utils import run_bass_kernel_spmd

NCORES = 8
B = 4096
D = 256
TWO_D = 2 * D  # 512 features
R = 128  # relations
BPC = B // NCORES  # 512 batch rows per core
KCH = TWO_D // 128  # 4 feature chunks of 128
LOG_SQRT_2PI = 0.9189385332046727

F32 = mybir.dt.float32
F16 = mybir.dt.float16

N_WARMUP = 16

_NC_CACHE = {}


def _np_dt(mm_dt):
    return np.float16 if mm_dt == F16 else np.float32


def _build_nc(mm_dt):
    nc = bacc.Bacc("TRN2", target_bir_lowering=False, debug=False)

    # Host-swizzled, SBUF-layout inputs (partition-major; contiguous DMAs):
    #   xt[p, k*BPC + b] = x[core_batch_off + b, k*128 + p]
    #   w [p, k*R + r]   = W[k*128 + p, r]   (k 0..3 x-coeffs, 4..7 x^2-coeffs)
    xt = nc.dram_tensor("xt", [128, KCH * BPC], mm_dt, kind="ExternalInput")
    w = nc.dram_tensor("w", [128, 2 * KCH * R], mm_dt, kind="ExternalInput")
    cvec = nc.dram_tensor("cvec", [R, 1], F32, kind="ExternalInput")
    out = nc.dram_tensor("out", [R, BPC], F32, kind="ExternalOutput")

    with tile.TileContext(nc) as tc:
        with (
            tc.tile_pool(name="const", bufs=1) as const,
            tc.tile_pool(name="data", bufs=1) as data,
            tc.tile_pool(name="psum", bufs=1, space="PSUM") as psum,
            tc.tile_pool(name="wpsum", bufs=1, space="PSUM") as wpsum_pool,
        ):
            xt_sb = data.tile([128, KCH, BPC], mm_dt)
            sq_sb = data.tile([128, KCH, BPC], mm_dt)
            w_sb = const.tile([128, 2 * KCH, R], mm_dt)
            c_sb = const.tile([R, 1], F32)

            # Input DMAs spread over both HWDGE queues (SP=sync, ACT=scalar)
            # so transfers overlap instead of serializing on one ring.
            for k in range(KCH):
                nc.sync.dma_start(
                    xt_sb[:, k, :], xt.ap()[:, k * BPC : (k + 1) * BPC]
                )
            nc.scalar.dma_start(
                w_sb[:, 0 : 2 * KCH, :], w.ap()[:, : 2 * KCH * R]
            )
            nc.scalar.dma_start(c_sb[:], cvec.ap())

            # PE warmup: the HAM clock gate holds the PE at 1.2 GHz until it
            # has been busy a while. Dummy matmuls on a memset tile during the
            # DMA wait raise the clock before the real matmuls issue.
            wdt = F32 if mm_dt == mybir.dt.float32r else mm_dt
            warm = const.tile([128, 128], wdt)
            nc.gpsimd.memset(warm[:], 0.0)
            wps = wpsum_pool.tile([1, 128], F32)
            for _ in range(N_WARMUP):
                nc.tensor.matmul(wps[:], warm[:, 0:1], warm[:], start=True, stop=True)

            for k in range(KCH):
                nc.vector.tensor_mul(sq_sb[:, k, :], xt_sb[:, k, :], xt_sb[:, k, :])

            ps = psum.tile([R, BPC], F32)
            for k in range(KCH):
                nc.tensor.matmul(
                    ps[:],
                    w_sb[:, k, :],
                    xt_sb[:, k, :],
                    start=(k == 0),
                    stop=False,
                )
            for k in range(KCH):
                nc.tensor.matmul(
                    ps[:],
                    w_sb[:, KCH + k, :],
                    sq_sb[:, k, :],
                    start=False,
                    stop=(k == KCH - 1),
                )

            # Evict + add c in two halves; store halves on separate queues so
            # the second add overlaps the first store.
            out_sb = data.tile([R, BPC], F32)
            half = BPC // 2
            nc.vector.tensor_scalar_add(out_sb[:, :half], ps[:, :half], c_sb[:])
            nc.sync.dma_start(out.ap()[:, :half], out_sb[:, :half])
            nc.vector.tensor_scalar_add(out_sb[:, half:], ps[:, half:], c_sb[:])
            nc.scalar.dma_start(out.ap()[:, half:], out_sb[:, half:])

    nc.compile()
    return nc


def _prepare(sbjs, objs, mus, sigmas, relation_priors, mm_dt):
    """Host-side parameter folding + batch sharding. Returns per-core in_maps."""
    np_dt = _np_dt(mm_dt)

    mus64 = mus.astype(np.float64)
    sig64 = sigmas.astype(np.float64)
    sig2 = sig64 * sig64
    wx = mus64 / sig2  # [R, 2D]
    wsq = -0.5 / sig2  # [R, 2D]
    c = (
        (-0.5 * mus64 * mus64 / sig2 - np.log(sig64) - LOG_SQRT_2PI).sum(axis=1)
        + relation_priors.astype(np.float64) * TWO_D
    )

    w_full = np.concatenate([wx.T, wsq.T], axis=0)  # [2*2D, R] d-major
    # swizzle to SBUF layout [p, chunk*R]
    w_sw = np.ascontiguousarray(
        w_full.reshape(2 * KCH, 128, R).transpose(1, 0, 2).reshape(128, 2 * KCH * R)
    ).astype(np_dt)
    c32 = np.ascontiguousarray(c.astype(np.float32).reshape(R, 1))

    x = np.concatenate([sbjs, objs], axis=1).astype(np_dt)  # [B, 2D]

    in_maps = []
    for i in range(NCORES):
        xp = x[i * BPC : (i + 1) * BPC]  # [BPC, 2D]
        # [b, k, p] -> [p, k, b] -> [128, KCH*BPC]
        xt_i = np.ascontiguousarray(
            xp.reshape(BPC, KCH, 128).transpose(2, 1, 0).reshape(128, KCH * BPC)
        )
        in_maps.append({"xt": xt_i, "w": w_sw, "cvec": c32})
    return in_maps


def run(sbjs, objs, mus, sigmas, relation_priors, mm_dt=F16, **run_kwargs):
    """Build (cached), run on 8 cores, gather. Returns (out [B, R] f32, results)."""
    key = str(mm_dt)
    if key not in _NC_CACHE:
        _NC_CACHE[key] = _build_nc(mm_dt)
    nc = _NC_CACHE[key]

    in_maps = _prepare(sbjs, objs, mus, sigmas, relation_priors, mm_dt)
    res = run_bass_kernel_spmd(nc, in_maps, core_ids=list(range(NCORES)), **run_kwargs)

    out = np.empty((B, R), dtype=np.float32)
    for i in range(NCORES):
        out[i * BPC : (i + 1) * BPC, :] = res.results[i]["out"].T
    return out, res


def kernel(sbjs, objs, mus, sigmas, relation_priors):
    out, _ = run(sbjs, objs, mus, sigmas, relation_priors)
    return out


if __name__ == "__main__":
    rng = np.random.default_rng(0)
    ins = {
        "sbjs": rng.standard_normal((B, D)).astype(np.float32),
        "objs": rng.standard_normal((B, D)).astype(np.float32),
        "mus": rng.standard_normal((R, TWO_D)).astype(np.float32),
        "sigmas": (np.abs(rng.standard_normal((R, TWO_D))) + 1.0).astype(np.float32),
        "relation_priors": rng.standard_normal((R,)).astype(np.float32),
    }
    out = kernel(**ins)
    print("out", out.shape, out.dtype, float(np.abs(out).max()))


# revision 12
# speedup vs baseline: 1.1474x; 1.0124x over previous
"""Trainium2 Bass kernel for nn_NaiveBayes (Gaussian naive-Bayes relation scorer).

Reference computes, for x = concat(sbjs, objs) [B, 2D]:
    out[b, r] = sum_d[ -0.5*((x_bd - mu_rd)/sig_rd)^2 - log(sig_rd) - LOG_SQRT_2PI ]
                + prior_r * 2D

Expanded into a matmul (per relation r, feature d):
    out[b, r] = sum_d x_bd * Wx[d, r] + sum_d (x_bd^2) * Wsq[d, r] + c_r
      Wx[d, r]  = mu_rd / sig_rd^2
      Wsq[d, r] = -0.5 / sig_rd^2
      c_r       = sum_d(-0.5*mu^2/sig^2 - log sig - LOG_SQRT_2PI) + prior_r * 2D

Sharding: data-parallel over batch: 4096 rows -> 8 cores x 512 rows.
mus/sigmas/priors fold host-side into W and c, replicated to all cores.

The x / W streams ship as fp16: fp16's 10 mantissa bits match what the PE's
fp32r (TF32) mode keeps anyway (measured 1.40e-5 vs 1.44e-5 scale-relative
absmax), at half the HBM bytes and full PE rate. Accumulation is fp32 PSUM;
c is added in fp32.

Host pre-swizzles both streams into the exact SBUF layout (partition-major,
[128, chunk*free]) so every DMA is a contiguous line-rate copy; X is also
pre-transposed to [d, b] (f32/fp16 DMA-transpose is unsupported / 2-byte-only
and this is free on the host during sharding). Each core computes
out^T [128 r, 512 b]: 8 accumulating PE matmuls (K = 8 x 128 chunks:
x-stream then x^2-stream), squares on DVE, c added during PSUM eviction in
two halves overlapped with the two output DMAs on separate HWDGE queues.
Host transposes + concatenates the 8 blocks.
"""

import numpy as np

import concourse.bacc as bacc
import concourse.tile as tile
from concourse import mybir
from concourse.bass_utils import run_bass_kernel_spmd

NCORES = 8
B = 4096
D = 256
TWO_D = 2 * D  # 512 features
R = 128  # relations
BPC = B // NCORES  # 512 batch rows per core
KCH = TWO_D // 128  # 4 feature chunks of 128
LOG_SQRT_2PI = 0.9189385332046727

F32 = mybir.dt.float32
F16 = mybir.dt.float16

N_WARMUP = 8

_NC_CACHE = {}


def _np_dt(mm_dt):
    return np.float16 if mm_dt == F16 else np.float32


def _build_nc(mm_dt):
    nc = bacc.Bacc("TRN2", target_bir_lowering=False, debug=False)

    # Host-swizzled, SBUF-layout inputs (partition-major; contiguous DMAs):
    #   xt[p, k*BPC + b] = x[core_batch_off + b, k*128 + p]
    #   w [p, k*R + r]   = W[k*128 + p, r]   (k 0..3 x-coeffs, 4..7 x^2-coeffs)
    xt = nc.dram_tensor("xt", [128, KCH * BPC], mm_dt, kind="ExternalInput")
    w = nc.dram_tensor("w", [128, 2 * KCH * R], mm_dt, kind="ExternalInput")
    cvec = nc.dram_tensor("cvec", [R, 1], F32, kind="ExternalInput")
    out = nc.dram_tensor("out", [R, BPC], F32, kind="ExternalOutput")

    with tile.TileContext(nc) as tc:
        with (
            tc.tile_pool(name="const", bufs=1) as const,
            tc.tile_pool(name="data", bufs=1) as data,
            tc.tile_pool(name="psum", bufs=1, space="PSUM") as psum,
            tc.tile_pool(name="wpsum", bufs=1, space="PSUM") as wpsum_pool,
        ):
            xt_sb = data.tile([128, KCH, BPC], mm_dt)
            sq_sb = data.tile([128, KCH, BPC], mm_dt)
            w_sb = const.tile([128, 2 * KCH, R], mm_dt)
            c_sb = const.tile([R, 1], F32)

            # Input DMAs spread over both HWDGE queues (SP=sync, ACT=scalar)
            # so transfers overlap instead of serializing on one ring. HWDGE
            # issue cost scales with descriptor (=partition) count, not bytes,
            # so fewer bigger DMAs issue faster; xt goes as two half-tensors
            # so compute on the first half starts one receipt-latency earlier.
            # cvec rides SWDGE: it is tiny and only needed by the final adds.
            half_x = KCH // 2
            nc.sync.dma_start(xt_sb[:, :half_x, :], xt.ap()[:, : half_x * BPC])
            nc.sync.dma_start(xt_sb[:, half_x:, :], xt.ap()[:, half_x * BPC :])
            nc.scalar.dma_start(w_sb[:, 0 : 2 * KCH, :], w.ap()[:, : 2 * KCH * R])
            nc.gpsimd.dma_start(c_sb[:], cvec.ap())

            # PE warmup: the HAM clock gate holds the PE at 1.2 GHz until it
            # has been busy ~3.4us within its activity window. Dummy matmuls
            # on a memset tile during the DMA wait raise the clock to 2.4 GHz
            # before the real matmuls issue.
            wdt = F32 if mm_dt == mybir.dt.float32r else mm_dt
            warm = const.tile([128, 512], wdt)
            nc.gpsimd.memset(warm[:], 0.0)
            wps = wpsum_pool.tile([1, 512], F32)
            for _ in range(N_WARMUP):
                nc.tensor.matmul(wps[:], warm[:, 0:1], warm[:], start=True, stop=True)

            for k in range(KCH):
                nc.vector.tensor_mul(sq_sb[:, k, :], xt_sb[:, k, :], xt_sb[:, k, :])

            ps = psum.tile([R, BPC], F32)
            for k in range(KCH):
                nc.tensor.matmul(
                    ps[:],
                    w_sb[:, k, :],
                    xt_sb[:, k, :],
                    start=(k == 0),
                    stop=False,
                )
            for k in range(KCH):
                nc.tensor.matmul(
                    ps[:],
                    w_sb[:, KCH + k, :],
                    sq_sb[:, k, :],
                    start=False,
                    stop=(k == KCH - 1),
                )

            # Evict + add c in two halves; store halves on separate queues so
            # the second add overlaps the first store.
            out_sb = data.tile([R, BPC], F32)
            half = BPC // 2
            nc.vector.tensor_scalar_add(out_sb[:, :half], ps[:, :half], c_sb[:])
            nc.sync.dma_start(out.ap()[:, :half], out_sb[:, :half])
            nc.vector.tensor_scalar_add(out_sb[:, half:], ps[:, half:], c_sb[:])
            nc.scalar.dma_start(out.ap()[:, half:], out_sb[:, half:])

    nc.compile()
    return nc


def _prepare(sbjs, objs, mus, sigmas, relation_priors, mm_dt):
    """Host-side parameter folding + batch sharding. Returns per-core in_maps."""
    np_dt = _np_dt(mm_dt)

    mus64 = mus.astype(np.float64)
    sig64 = sigmas.astype(np.float64)
    sig2 = sig64 * sig64
    wx = mus64 / sig2  # [R, 2D]
    wsq = -0.5 / sig2  # [R, 2D]
    c = (
        (-0.5 * mus64 * mus64 / sig2 - np.log(sig64) - LOG_SQRT_2PI).sum(axis=1)
        + relation_priors.astype(np.float64) * TWO_D
    )

    w_full = np.concatenate([wx.T, wsq.T], axis=0)  # [2*2D, R] d-major
    # swizzle to SBUF layout [p, chunk*R]
    w_sw = np.ascontiguousarray(
        w_full.reshape(2 * KCH, 128, R).transpose(1, 0, 2).reshape(128, 2 * KCH * R)
    ).astype(np_dt)
    c32 = np.ascontiguousarray(c.astype(np.float32).reshape(R, 1))

    x = np.concatenate([sbjs, objs], axis=1).astype(np_dt)  # [B, 2D]

    in_maps = []
    for i in range(NCORES):
        xp = x[i * BPC : (i + 1) * BPC]  # [BPC, 2D]
        # [b, k, p] -> [p, k, b] -> [128, KCH*BPC]
        xt_i = np.ascontiguousarray(
            xp.reshape(BPC, KCH, 128).transpose(2, 1, 0).reshape(128, KCH * BPC)
        )
        in_maps.append({"xt": xt_i, "w": w_sw, "cvec": c32})
    return in_maps


def run(sbjs, objs, mus, sigmas, relation_priors, mm_dt=F16, **run_kwargs):
    """Build (cached), run on 8 cores, gather. Returns (out [B, R] f32, results)."""
    key = str(mm_dt)
    if key not in _NC_CACHE:
        _NC_CACHE[key] = _build_nc(mm_dt)
    nc = _NC_CACHE[key]

    in_maps = _prepare(sbjs, objs, mus, sigmas, relation_priors, mm_dt)
    res = run_bass_kernel_spmd(nc, in_maps, core_ids=list(range(NCORES)), **run_kwargs)

    out = np.empty((B, R), dtype=np.float32)
    for i in range(NCORES):
        out[i * BPC : (i + 1) * BPC, :] = res.results[i]["out"].T
    return out, res


def kernel(sbjs, objs, mus, sigmas, relation_priors):
    out, _ = run(sbjs, objs, mus, sigmas, relation_priors)
    return out


if __name__ == "__main__":
    rng = np.random.default_rng(0)
    ins = {
        "sbjs": rng.standard_normal((B, D)).astype(np.float32),
        "objs": rng.standard_normal((B, D)).astype(np.float32),
        "mus": rng.standard_normal((R, TWO_D)).astype(np.float32),
        "sigmas": (np.abs(rng.standard_normal((R, TWO_D))) + 1.0).astype(np.float32),
        "relation_priors": rng.standard_normal((R,)).astype(np.float32),
    }
    out = kernel(**ins)
    print("out", out.shape, out.dtype, float(np.abs(out).max()))
